# revision 2
# baseline (speedup 1.0000x reference)
"""Trainium2 Bass kernel v6: sharded weights + device AllGather.

Measured reality on this axon-tunneled setup: per-exec cost is dominated
by INPUT STAGING at ~0.85ms per MB of per-core input bytes; compute is
nearly free (L1 vs L3 ablation shows ~0.6ms/layer). AllGather of 11MB
costs ~0.9ms. So v6 ships the replicated tensors (wm bf16, wm8 dual-fp8,
nf gather table) SHARDED 1/8 per core and reassembles them on device with
three DRAM AllGathers — full numeric precision, ~11.4MB fewer input bytes
per core. x0 is cut (gathered from the assembled nf table); the big const
blob is split to two small rows.

Compute structure is v4's (gather-once nf table in e4m3, resident edges,
quarter-tile dual-fp8 message MLP, 2-group pipelined tails).
"""
import numpy as np
import ml_dtypes
import concourse.bass as bass
import concourse.bacc as bacc
import concourse.mybir as mybir
from concourse import tile
from concourse.bass_utils import run_bass_kernel_spmd
from contextlib import ExitStack

F32 = mybir.dt.float32
F32R = mybir.dt.float32r
BF16 = mybir.dt.bfloat16
I16 = mybir.dt.int16
I32 = mybir.dt.int32
F8 = mybir.dt.float8e4
AF = mybir.ActivationFunctionType
OP = mybir.AluOpType
DRM = mybir.MatmulPerfMode.DoubleRow
BF = ml_dtypes.bfloat16

N, K, NF, L = 2048, 48, 384, 3
NCORES = 8
NLOC = N // NCORES            # 256
NG = 128                      # nodes per group
TG = NG * K                   # 6144 tokens per group (k-major: t = k*128 + n)
GC = 1536                     # gather/edge chunk (tokens)
NGC = TG // GC                # 4 chunks per group
SCALE = 30.0
EPS = 1e-5
MAGIC = 0x5F3759DF

O_W0X = 0
O_W2 = 1152
O_DW0 = 2304
O_DW1 = 6912
WMC = 11520
O8_W0E = 0
O8_W1 = 2304
O8_W0N = 4608
W8C = 6912
SC_W0E = 32.0
SC_W1 = 16.0
SC_W0N = 32.0
O_B0 = 0
O_B1 = 3
O_B2 = 6
O_DB0 = 9
O_DB1 = 21

SL3 = (slice(0, 2), slice(1, 3), slice(0, 3, 2))

# single AllGather blob: per-core bytes = nf shard | wm shard | wm8 shard
NFB_B = NLOC * 384 * 2            # 196608
WMSH_B = L * 16 * WMC * 2         # 1105920
WM8SH_B = L * 16 * W8C            # 331776
SHB = NFB_B + WMSH_B + WM8SH_B    # 1634304

_NC_CACHE = {}


def _emit(act=None, layers=L):
    act = AF.Gelu if act is None else act
    nc = bacc.Bacc(num_swdge_queues=4)
    edge_p = nc.declare_dram_parameter("edge", [128, 2, NGC, 3, GC], F8,
                                       isOutput=False)
    wsh_p = nc.declare_dram_parameter("wsh", [1, SHB], mybir.dt.uint8,
                                      isOutput=False)
    gidx_p = nc.declare_dram_parameter("gidx", [128, 2, NGC, 3, 32], I16,
                                       isOutput=False)
    x0i_p = nc.declare_dram_parameter("x0i", [128, 16], I16, isOutput=False)
    x0_p = nc.declare_dram_parameter("x0", [128, 3, NLOC], F32R, isOutput=False)
    wb_p = nc.declare_dram_parameter("wb", [L, 128, 24], F32, isOutput=False)
    ln_p = nc.declare_dram_parameter("lnpk", [L, 1, 1920], F32R, isOutput=False)
    b1r_p = nc.declare_dram_parameter("b1r", [L, 1, 384], F32R, isOutput=False)
    crow_p = nc.declare_dram_parameter("crow", [1, 769], F32R, isOutput=False)
    ccol_p = nc.declare_dram_parameter("ccol", [128, 8], F32R, isOutput=False)
    cstb_p = nc.declare_dram_parameter("constsb", [128, 128], BF16, isOutput=False)
    cst8_p = nc.declare_dram_parameter("consts8", [128, 2, 128], F8, isOutput=False)
    mask_p = nc.declare_dram_parameter("mask", [1, NLOC], F32, isOutput=False)
    out_p = nc.declare_dram_parameter("out_x", [128, 3, NLOC], F32, isOutput=True)

    with tile.TileContext(nc) as tc, ExitStack() as ctx:
        wpool = ctx.enter_context(tc.tile_pool(name="w", bufs=2))
        gpool = ctx.enter_context(tc.tile_pool(name="g", bufs=2))
        nfpool = ctx.enter_context(tc.tile_pool(name="nf8", bufs=1))
        epool = ctx.enter_context(tc.tile_pool(name="ep", bufs=1))
        work1 = ctx.enter_context(tc.tile_pool(name="work1", bufs=1))
        work2 = ctx.enter_context(tc.tile_pool(name="work2", bufs=2))
        xpool = ctx.enter_context(tc.tile_pool(name="xp", bufs=2))
        small = ctx.enter_context(tc.tile_pool(name="small", bufs=1))
        dram = ctx.enter_context(tc.tile_pool(name="dram", bufs=1, space="DRAM"))
        mm = ctx.enter_context(tc.tile_pool(name="mm", bufs=3, space="PSUM"))
        aggp = ctx.enter_context(tc.tile_pool(name="aggp", bufs=1, space="PSUM"))
        tpp = ctx.enter_context(tc.tile_pool(name="tpp", bufs=1, space="PSUM"))

        RG = [list(range(NCORES))]

        # --- shard staging + ONE AllGather (multiple concurrent collectives
        # complete out of order vs the shared Collectives sem -> consumers
        # could read in-flight data; a single collective has no such window)
        blob_st = dram.tile([1, SHB], mybir.dt.uint8, tag="bst")
        nc.sync.dma_start(blob_st[:], wsh_p[:])
        blob = dram.tile([NCORES, SHB], mybir.dt.uint8, tag="blob")
        nc.gpsimd.collective_compute(
            "AllGather", mybir.AluOpType.bypass, replica_groups=RG,
            ins=[blob_st[:]], outs=[blob[:]])
        flat = blob[:].rearrange("a s -> (a s)")

        # repack the nf gather table to contiguous node-major [N, 384]
        nfbfull = dram.tile([N, 384], BF16, tag="nfbf")
        for cc in range(NCORES):
            nc.sync.dma_start(
                nfbfull[cc * NLOC:(cc + 1) * NLOC, :],
                flat[cc * SHB:cc * SHB + NFB_B].bitcast(BF16)
                .rearrange("(n e) -> n e", n=NLOC))

        def wm_view(cc, l):
            o = cc * SHB + NFB_B + l * 16 * WMC * 2
            return (flat[o:o + 16 * WMC * 2].bitcast(BF16)
                    .rearrange("(q w) -> q w", q=16))

        def wm8_view(cc, l):
            o = cc * SHB + NFB_B + WMSH_B + l * 16 * W8C
            return (flat[o:o + 16 * W8C].bitcast(F8)
                    .rearrange("(q w) -> q w", q=16))

        # --- one-time loads ---
        crow = small.tile([1, 769], F32R, tag="crow")
        nc.sync.dma_start(crow[:], crow_p[:])
        ccol = small.tile([128, 8], F32R, tag="ccol")
        nc.sync.dma_start(ccol[:], ccol_p[:])
        cstb = small.tile([128, 128], BF16, tag="cstb")
        nc.sync.dma_start(cstb[:], cstb_p[:])
        cst8 = small.tile([128, 2, 128], F8, tag="cst8")
        nc.sync.dma_start(cst8[:], cst8_p[:])
        gidx = small.tile([128, 2, NGC, 3, 32], I16, tag="gidx")
        nc.sync.dma_start(gidx[:], gidx_p[:])
        x0i = small.tile([128, 16], I16, tag="x0i")
        nc.sync.dma_start(x0i[:], x0i_p[:])
        maskt = small.tile([1, NLOC], F32, tag="maskt")
        nc.sync.dma_start(maskt[:], mask_p[:])

        ones_col = ccol[:, 0:1]           # [128,1] ones (stats lhsT)
        ones_row = crow[0:1, 0:128]       # [1,128] ones
        ones512 = crow[0:1, 0:512]        # [1,512] ones
        eye_b = cstb[:]                   # [128,128] identity*SC_W0E bf16
        eye2 = cst8[:]                    # [128,2,128] identity pair fp8e4

        xg = {}      # (g) -> current residual tile [128,3,128] f32r
        xbg = {}     # bf16 copy for matmul rhs
        for g in range(2):
            xt = xpool.tile([128, 3, NG], F32R, tag=f"x{g}", name=f"x{g}")
            nc.sync.dma_start(xt[:], x0_p[:, :, g * NG:(g + 1) * NG])
            xg[g] = xt
            xbt = xpool.tile([128, 3, NG], BF16, tag=f"xb{g}", name=f"xb{g}")
            nc.vector.tensor_copy(xbt[:], xt[:].bitcast(F32))
            xbg[g] = xbt

        def load_weights(l):
            wm = wpool.tile([128, WMC], BF16, tag="wm", name=f"wm{l}")
            for cc in range(NCORES):
                nc.scalar.dma_start(wm[cc * 16:(cc + 1) * 16, :],
                                    wm_view(cc, l))
            wm8 = wpool.tile([128, W8C], F8, tag="wm8", name=f"wm8{l}")
            for cc in range(NCORES):
                nc.scalar.dma_start(wm8[cc * 16:(cc + 1) * 16, :],
                                    wm8_view(cc, l))
            wb = wpool.tile([128, 24], F32, tag="wb", name=f"wb{l}")
            nc.scalar.dma_start(wb[:], wb_p[l])
            lnw = wpool.tile([1, 1920], F32R, tag="lnw", bufs=1,
                             name=f"lnw{l}")
            nc.scalar.dma_start(lnw[:], ln_p[l])
            b1r = wpool.tile([1, 384], F32R, tag="b1r", name=f"b1r{l}")
            nc.scalar.dma_start(b1r[:], b1r_p[l])
            return wm, wm8, wb, lnw, b1r

        wms = {0: load_weights(0)}

        # --- gather nf0[idx] once (24 x 512-idx), convert to e4m3 ---
        nf8 = {}
        for g in range(2):
            for c in range(NGC):
                t8 = nfpool.tile([128, 3, GC], F8, tag=f"nf8_{g}_{c}",
                                 name=f"nf8_{g}_{c}")
                for h in range(GC // 512):
                    stage = gpool.tile([128, 3, 512], BF16, tag="st",
                                       name=f"st{g}{c}{h}")
                    nc.gpsimd.dma_gather(
                        stage[:], nfbfull[:], gidx[:, g, c, h, :],
                        num_idxs=512, num_idxs_reg=512, elem_size=384,
                        transpose=True,
                        queue_num=(g * NGC * 3 + c * 3 + h) % 4)
                    nc.vector.tensor_copy(
                        t8[:, :, h * 512:(h + 1) * 512], stage[:])
                nf8[(g, c)] = t8

        # --- edge features resident in SBUF, split per (g, c) ---
        et_all = epool.tile([128, 2, NGC, 3, GC], F8, tag="et")
        for g in range(2):
            for c in range(NGC):
                nc.sync.dma_start(et_all[:, g, c], edge_p[:, g, c])

        def prep_xw(l, g, wm, wb):
            """xw2 = dup2(x_g @ W0x_l + b0) bf16 [128,3,2,128]."""
            xwp = tpp.tile([128, 384], F32, tag="tp", name="xwp")
            for mt in range(3):
                for kt in range(3):
                    nc.tensor.matmul(
                        xwp[:, mt * NG:(mt + 1) * NG],
                        wm[:, O_W0X + kt * 384 + mt * 128: O_W0X + kt * 384 + (mt + 1) * 128],
                        xbg[g][:, kt, :],
                        start=(kt == 0), stop=(kt == 2))
            xw2 = work1.tile([128, 3, 2, NG], BF16, tag=f"xw2{g}",
                             name=f"xw2{g}")
            for mt in range(3):
                nc.vector.tensor_scalar(
                    xw2[:, mt, 0, :], xwp[:, mt * NG:(mt + 1) * NG],
                    wb[:, O_B0 + mt:O_B0 + mt + 1], None, op0=OP.add)
            nc.vector.tensor_copy(xw2[:, :, 1, :], xw2[:, :, 0, :])
            return xw2

        def kloop(l, g, xw2, wm8, b1r):
            """Accumulate hsum = sum_k h1 for group g. Returns PSUM tile."""
            hsum = aggp.tile([128, 384], F32, tag="agg", name=f"hs{l}{g}")
            for c in range(NGC):
                for qq in range(GC // 256):
                    off = qq * 256
                    h0g = work2.tile([128, 3, 256], F8, tag="h0g", name="h0g")
                    hp = mm.tile([128, 3, 256], F32, tag="mm", name="hp")
                    for mt in range(3):
                        for p8, sl in enumerate(SL3):
                            nc.tensor.matmul(
                                hp[:, mt, :],
                                wm8[:, O8_W0E + mt * 768 + p8 * 256:
                                    O8_W0E + mt * 768 + (p8 + 1) * 256]
                                .rearrange("p (a b) -> p a b", a=2),
                                et_all[:, g, c, sl, off:off + 256],
                                start=(p8 == 0), stop=False, perf_mode=DRM)
                        for p8, sl in enumerate(SL3):
                            nc.tensor.matmul(
                                hp[:, mt, :],
                                wm8[:, O8_W0N + mt * 768 + p8 * 256:
                                    O8_W0N + mt * 768 + (p8 + 1) * 256]
                                .rearrange("p (a b) -> p a b", a=2),
                                nf8[(g, c)][:, sl, off:off + 256],
                                start=False, stop=False, perf_mode=DRM)
                        nc.tensor.matmul(
                            hp[:, mt, :], eye_b,
                            xw2[:, mt, :, :].rearrange("p a b -> p (a b)"),
                            start=False, stop=True)
                    nc.scalar.activation(h0g[:], hp[:], act,
                                         scale=1.0 / SC_W0E)
                    h1g = work2.tile([128, 3, 256], F8, tag="h1g", name="h1g")
                    h1p = mm.tile([128, 3, 256], F32, tag="mm", name="h1p")
                    for mt in range(3):
                        nc.tensor.matmul(
                            h1p[:, mt, :],
                            b1r[0:1, mt * 128:(mt + 1) * 128],
                            ones512[0:1, 0:256], start=True, stop=False)
                        for p8, sl in enumerate(SL3):
                            nc.tensor.matmul(
                                h1p[:, mt, :],
                                wm8[:, O8_W1 + mt * 768 + p8 * 256:
                                    O8_W1 + mt * 768 + (p8 + 1) * 256]
                                .rearrange("p (a b) -> p a b", a=2),
                                h0g[:, sl, :],
                                start=False, stop=(p8 == 2), perf_mode=DRM)
                    nc.scalar.activation(h1g[:], h1p[:], act,
                                         scale=1.0 / SC_W1)
                    first = (c == 0 and qq == 0)
                    last = (c == NGC - 1 and qq == GC // 256 - 1)
                    for mt in range(3):
                        nc.tensor.matmul(
                            hsum[:, mt * NG:(mt + 1) * NG],
                            eye2,
                            h1g[:, mt, :]
                            .rearrange("p (a b) -> p a b", a=2),
                            start=(first and mt == 0),
                            stop=last,
                            perf_mode=DRM,
                            skip_group_check=True)
            return hsum

        def rsqrt_row(v):
            """[1,n] f32 SBUF -> [1,n] f32 rstd, DVE-only Newton iteration."""
            n = v.shape[-1]
            yi = small.tile([1, n], F32, tag="yi", name="yi")
            tn = small.tile([1, n], F32, tag="tn", name="tn")
            nc.vector.tensor_scalar(
                yi[:].bitcast(I32), v[:].bitcast(I32), 1, None,
                op0=OP.logical_shift_right)
            nc.vector.tensor_copy(tn[:], yi[:].bitcast(I32))
            nc.vector.tensor_scalar(tn[:], tn[:], -1.0, float(MAGIC),
                                    op0=OP.mult, op1=OP.add)
            nc.vector.tensor_copy(yi[:].bitcast(I32), tn[:])
            y = yi[:].bitcast(F32)
            for _ in range(2):
                nc.vector.tensor_mul(tn[:], y, y)
                nc.vector.tensor_mul(tn[:], tn[:], v[:])
                nc.vector.tensor_scalar(tn[:], tn[:], -0.5, 1.5,
                                        op0=OP.mult, op1=OP.add)
                nc.vector.tensor_mul(y, y, tn[:])
            return yi

        def layernorm(src, lnw, ln_i, g, masked, tag, tp_tile):
            """src: [128,3,128] F32R tile -> new [128,3,128] f32r tile."""
            maskg = maskt[0:1, g * NG:(g + 1) * NG]
            sq = work1.tile([128, 3, NG], F32R, tag="sq", name="sq")
            nc.vector.tensor_mul(sq[:], src[:].bitcast(F32), src[:].bitcast(F32))
            st = tp_tile("st")
            for kt in range(3):
                nc.tensor.matmul(st[0:1, 0:NG], ones_col, src[:, kt, :],
                                 start=(kt == 0), stop=(kt == 2))
            for kt in range(3):
                nc.tensor.matmul(st[0:1, NG:2 * NG], ones_col, sq[:, kt, :],
                                 start=(kt == 0), stop=(kt == 2))
            sm = small.tile([1, 2 * NG], F32, tag="sm", name="sm")
            nc.vector.tensor_scalar_mul(sm[:], st[0:1, 0:2 * NG], 1.0 / NF)
            var = small.tile([1, NG], F32, tag="var", name="var")
            nc.vector.tensor_mul(var[:], sm[0:1, 0:NG], sm[0:1, 0:NG])
            nc.vector.tensor_sub(var[:], sm[0:1, NG:2 * NG], var[:])
            nc.vector.tensor_scalar_add(var[:], var[:], EPS)
            rstd = rsqrt_row(var)
            rv = small.tile([1, 384], F32R, tag="rv", name="rv")
            nmr = small.tile([1, NG], F32, tag="nmr", name="nmr")
            nc.vector.tensor_scalar(nmr[:], sm[0:1, 0:NG], -1.0, None,
                                    op0=OP.mult)
            nc.vector.tensor_mul(nmr[:], nmr[:], rstd[:].bitcast(F32))
            if masked:
                nc.vector.tensor_mul(rv[0:1, 0:NG], rstd[:].bitcast(F32), maskg)
                nc.vector.tensor_mul(rv[0:1, NG:2 * NG], nmr[:], maskg)
                nc.vector.tensor_copy(rv[0:1, 2 * NG:3 * NG], maskg)
            else:
                nc.vector.tensor_copy(rv[0:1, 0:NG], rstd[:].bitcast(F32))
                nc.vector.tensor_copy(rv[0:1, NG:2 * NG], nmr[:])
                nc.vector.tensor_copy(rv[0:1, 2 * NG:3 * NG],
                                      ones_row.bitcast(F32))
            outt = xpool.tile([128, 3, NG], F32R, tag=tag, name=tag)
            stp = tp_tile("stS")
            for mt in range(3):
                woff = ln_i * 384 + mt * 128
                nc.tensor.matmul(stp[:, mt * NG:(mt + 1) * NG],
                                 lnw[0:1, woff:woff + 128],
                                 rv[0:1, 0:NG], start=True, stop=True)
            nc.vector.tensor_mul(
                outt[:], src[:].bitcast(F32),
                stp[:].rearrange("p (a b) -> p a b", a=3))
            stp2 = tp_tile("stT")
            for mt in range(3):
                woff = ln_i * 384 + mt * 128
                nc.tensor.matmul(stp2[:, mt * NG:(mt + 1) * NG],
                                 lnw[0:1, 768 + woff:768 + woff + 128],
                                 rv[0:1, 2 * NG:3 * NG], start=True, stop=False)
                nc.tensor.matmul(stp2[:, mt * NG:(mt + 1) * NG],
                                 lnw[0:1, woff:woff + 128],
                                 rv[0:1, NG:2 * NG], start=False, stop=True)
            nc.vector.tensor_add(
                outt[:], outt[:].bitcast(F32),
                stp2[:].rearrange("p (a b) -> p a b", a=3))
            return outt

        def tail(l, g, hsum, wm, wb, lnw, wm_next, wb_next, final=False):

            def tp_tile(name):
                if final:
                    t = mm.tile([128, 3, 256], F32, tag="mm", name=name)
                    return t[:].rearrange("p a b -> p (a b)")[:, 0:384]
                return tpp.tile([128, 384], F32, tag="tp", name=name)
            hsum_s = work1.tile([128, 3, NG], BF16, tag="hsum_s", name="hsum_s")
            nc.vector.tensor_copy(
                hsum_s[:], hsum[:].rearrange("p (a b) -> p a b", a=3))
            aggm = tp_tile("aggm")
            for mt in range(3):
                for kt in range(3):
                    nc.tensor.matmul(
                        aggm[:, mt * NG:(mt + 1) * NG],
                        wm[:, O_W2 + kt * 384 + mt * 128: O_W2 + kt * 384 + (mt + 1) * 128],
                        hsum_s[:, kt, :],
                        start=(kt == 0), stop=(kt == 2))
            x1p = work1.tile([128, 3, NG], F32R, tag="x1p", name="x1p")
            for mt in range(3):
                nc.vector.tensor_scalar(
                    x1p[:, mt, :], aggm[:, mt * NG:(mt + 1) * NG],
                    1.0 / SCALE, b2s[:, mt:mt + 1], op0=OP.mult, op1=OP.add)
            nc.vector.tensor_add(x1p[:], x1p[:].bitcast(F32),
                                 xg[g][:].bitcast(F32))
            x1 = layernorm(x1p, lnw, 0, g, masked=False, tag=f"x1_{g}",
                           tp_tile=tp_tile)
            x1b = work1.tile([128, 3, NG], BF16, tag=f"x1b{g}", bufs=2,
                             name="x1b")
            nc.vector.tensor_copy(x1b[:], x1[:].bitcast(F32))

            d0g = work1.tile([128, 12, NG], BF16, tag="d0g", name="d0g")
            for r in range(4):
                dp = tp_tile("dp")
                for j in range(3):
                    mt = r * 3 + j
                    reg = dp[:, j * NG:(j + 1) * NG]
                    for kt in range(3):
                        nc.tensor.matmul(
                            reg,
                            wm[:, O_DW0 + kt * 1536 + mt * 128: O_DW0 + kt * 1536 + (mt + 1) * 128],
                            x1b[:, kt, :],
                            start=(kt == 0), stop=(kt == 2))
                    nc.scalar.activation(d0g[:, mt, :], reg, act,
                                         bias=wb[:, O_DB0 + mt:O_DB0 + mt + 1])
            d1p = tp_tile("d1p")
            for mt in range(3):
                for kt in range(12):
                    nc.tensor.matmul(
                        d1p[:, mt * NG:(mt + 1) * NG],
                        wm[:, O_DW1 + kt * 384 + mt * 128: O_DW1 + kt * 384 + (mt + 1) * 128],
                        d0g[:, kt, :],
                        start=(kt == 0), stop=(kt == 11))
            x2p = work1.tile([128, 3, NG], F32R, tag="x2p", name="x2p")
            for mt in range(3):
                nc.vector.tensor_scalar(
                    x2p[:, mt, :], d1p[:, mt * NG:(mt + 1) * NG],
                    1.0, wb[:, O_DB1 + mt:O_DB1 + mt + 1],
                    op0=OP.mult, op1=OP.add)
            nc.vector.tensor_add(x2p[:], x2p[:].bitcast(F32),
                                 x1[:].bitcast(F32))
            xo = layernorm(x2p, lnw, 1, g, masked=True, tag=f"x{g}",
                           tp_tile=tp_tile)
            xg[g] = xo
            if l + 1 < layers:
                xb = xpool.tile([128, 3, NG], BF16, tag=f"xb{g}", name=f"xb{g}")
                nc.vector.tensor_copy(xb[:], xo[:].bitcast(F32))
                xbg[g] = xb
                return prep_xw(l + 1, g, wm_next, wb_next)
            nc.sync.dma_start(out_p[:, :, g * NG:(g + 1) * NG],
                              xo[:].bitcast(F32))
            return None

        # ================= pipeline =================
        wms[1] = load_weights(1)
        b2s_all = {}

        def get_b2s(l, wb):
            if l not in b2s_all:
                t = small.tile([128, 3], F32, tag=f"b2s{l % 2}", name=f"b2s{l}")
                nc.vector.tensor_scalar_mul(t[:], wb[:, O_B2:O_B2 + 3],
                                            K / SCALE)
                b2s_all[l] = t
            return b2s_all[l]

        xw2s = {}
        wm0, _, wb0, _, _ = wms[0]
        for g in range(2):
            xw2s[g] = prep_xw(0, g, wm0, wb0)

        for l in range(layers):
            wm, wm8, wb, lnw, b1r = wms[l]
            b2s = get_b2s(l, wb)
            if l + 1 < layers:
                if l + 1 not in wms:
                    wms[l + 1] = load_weights(l + 1)
                wm_next, _, wb_next, _, _ = wms[l + 1]
            else:
                wm_next = wb_next = None
            for g in range(2):
                hsum = kloop(l, g, xw2s[g], wm8, b1r)
                xw2s[g] = tail(l, g, hsum, wm, wb, lnw, wm_next, wb_next,
                               final=(l == layers - 1 and g == 1))

    nc.finalize()
    return nc


def _get_nc():
    if "nc" not in _NC_CACHE:
        _NC_CACHE["nc"] = _emit()
    return _NC_CACHE["nc"]


def _fm(w):
    """[in, out] fp32 -> [128, n_kt*out] (feature-major lhsT blob columns)."""
    i, o = w.shape
    return np.ascontiguousarray(
        w.reshape(i // 128, 128, o).transpose(1, 0, 2).reshape(128, -1))


def _wrap_idx(vals):
    """[n] int -> [128, n//16] int16 wrapped (i -> [i%16, i//16]) x8 replicas."""
    n = vals.shape[0]
    w = np.ascontiguousarray(vals.reshape(n // 16, 16).T).astype(np.int16)
    return np.tile(w, (8, 1))


def _marshal(inputs):
    nf = np.asarray(inputs["node_features"], np.float32)
    ef = np.asarray(inputs["edge_features"], np.float32)
    idx = np.asarray(inputs["neighbor_indices"])
    mask = np.asarray(inputs["mask"], np.float32)

    f8np = mybir.dt.np(mybir.dt.float8e4)
    nfb = nf.astype(BF)                                    # [N,384] full table
    wm = np.empty((L, 128, WMC), BF)
    wm8 = np.empty((L, 128, W8C), f8np)
    wb = np.empty((L, 128, 24), np.float32)
    lnpk = np.empty((L, 1, 1920), np.float32)
    b1r_m = np.empty((L, 1, 384), np.float32)
    for l in range(L):
        w0 = np.asarray(inputs["msg_w0"], np.float32)[l]
        cols = [
            _fm(w0[0:384]),
            _fm(np.asarray(inputs["msg_w2"], np.float32)[l]),
            _fm(np.asarray(inputs["dense_w0"], np.float32)[l]),
            _fm(np.asarray(inputs["dense_w1"], np.float32)[l]),
        ]
        wm[l] = np.concatenate(cols, axis=1).astype(BF)
        w0e = _fm(w0[384:768])
        w1f = _fm(np.asarray(inputs["msg_w1"], np.float32)[l])
        w0n = _fm(w0[1152:1536])
        c8 = []
        for W, sc in ((w0e, SC_W0E), (w1f, SC_W1), (w0n, SC_W0N)):
            q = (W * sc).astype(f8np)
            d = (W * sc - q.astype(np.float32)).astype(f8np)
            for mt in range(3):
                blk = lambda A, kt: A[:, kt * 384 + mt * 128:
                                      kt * 384 + (mt + 1) * 128]
                c8 += [blk(q, 0), blk(q, 1), blk(d, 1), blk(d, 2),
                       blk(d, 0), blk(q, 2)]
        wm8[l] = np.concatenate(
            [c.astype(f8np) for c in c8], axis=1)
        bcols = [
            np.asarray(inputs["msg_b0"], np.float32)[l].reshape(3, 128).T,
            np.asarray(inputs["msg_b1"], np.float32)[l].reshape(3, 128).T,
            np.asarray(inputs["msg_b2"], np.float32)[l].reshape(3, 128).T,
            np.asarray(inputs["dense_b0"], np.float32)[l].reshape(12, 128).T,
            np.asarray(inputs["dense_b1"], np.float32)[l].reshape(3, 128).T,
        ]
        wb[l] = np.concatenate(bcols, axis=1)
        lnpk[l, 0] = np.concatenate([
            np.asarray(inputs["ln1_w"], np.float32)[l],
            np.asarray(inputs["ln2_w"], np.float32)[l],
            np.asarray(inputs["ln1_b"], np.float32)[l],
            np.asarray(inputs["ln2_b"], np.float32)[l],
            np.asarray(inputs["msg_b1"], np.float32)[l] * SC_W1])
        b1r_m[l, 0] = np.asarray(inputs["msg_b1"], np.float32)[l] * SC_W1
    crow = np.ones((1, 769), np.float32)
    ccol = np.ones((128, 8), np.float32)
    constsb = (np.eye(128, dtype=np.float32) * SC_W0E).astype(BF)
    consts8 = np.broadcast_to(np.eye(128, dtype=np.float32), (2, 128, 128))
    consts8 = np.ascontiguousarray(
        consts8.transpose(1, 0, 2)).astype(f8np)

    in_maps = []
    for c in range(NCORES):
        lo = slice(c * NLOC, (c + 1) * NLOC)
        efc = ef[lo]                                       # [256,48,384]
        idc = idx[lo]                                      # [256,48]
        edge = np.empty((128, 2, NGC, 3, GC), f8np)
        gidx = np.empty((128, 2, NGC, 3, 32), np.int16)
        for g in range(2):
            gs = slice(g * NG, (g + 1) * NG)
            E = efc[gs].transpose(1, 0, 2).reshape(TG, 384)    # k-major tokens
            idx_k = np.ascontiguousarray(idc[gs].T).reshape(TG)
            for cc in range(NGC):
                Ec = E[cc * GC:(cc + 1) * GC]
                edge[:, g, cc] = (Ec.reshape(GC, 3, 128)
                                  .transpose(2, 1, 0).astype(f8np))
                for h in range(3):
                    t0 = cc * GC + h * 512
                    gidx[:, g, cc, h] = _wrap_idx(idx_k[t0:t0 + 512])
        x0i = _wrap_idx(np.arange(c * NLOC, (c + 1) * NLOC))
        x0 = np.ascontiguousarray(
            nf[lo].reshape(NLOC, 3, 128).transpose(2, 1, 0))   # [128,3,256]
        wsh = np.concatenate([
            np.ascontiguousarray(nfb[lo]).view(np.uint8).reshape(-1),
            np.ascontiguousarray(
                wm[:, c * 16:(c + 1) * 16, :]).view(np.uint8).reshape(-1),
            np.ascontiguousarray(
                wm8[:, c * 16:(c + 1) * 16, :]).view(np.uint8).reshape(-1),
        ])[None, :]
        in_maps.append(dict(
            edge=edge, gidx=gidx,
            x0i=x0i, x0=x0, wsh=wsh,
            wb=wb, lnpk=lnpk, crow=crow, ccol=ccol,
            constsb=constsb, consts8=consts8,
            b1r=b1r_m,
            mask=np.ascontiguousarray(mask[lo])[None, :]))
    return in_maps


def _unshard(results):
    out = np.empty((N, NF), np.float32)
    for c in range(NCORES):
        xfm = results[c]["out_x"]                          # [128,3,256]
        out[c * NLOC:(c + 1) * NLOC] = xfm.transpose(2, 1, 0).reshape(NLOC, NF)
    return out


def kernel(**inputs):
    nc = _get_nc()
    in_maps = _marshal(inputs)
    res = run_bass_kernel_spmd(nc, in_maps, list(range(NCORES)), trace=False)
    return _unshard(res.results)


# revision 7
# speedup vs baseline: 1.0280x; 1.0280x over previous
"""Trainium2 Bass kernel v6: sharded weights + device AllGather.

Measured reality on this axon-tunneled setup: per-exec cost is dominated
by INPUT STAGING at ~0.85ms per MB of per-core input bytes; compute is
nearly free (L1 vs L3 ablation shows ~0.6ms/layer). AllGather of 11MB
costs ~0.9ms. So v6 ships the replicated tensors (wm bf16, wm8 dual-fp8,
nf gather table) SHARDED 1/8 per core and reassembles them on device with
three DRAM AllGathers — full numeric precision, ~11.4MB fewer input bytes
per core. x0 is cut (gathered from the assembled nf table); the big const
blob is split to two small rows.

Compute structure is v4's (gather-once nf table in e4m3, resident edges,
quarter-tile dual-fp8 message MLP, 2-group pipelined tails).
"""
import numpy as np
import ml_dtypes
import concourse.bass as bass
import concourse.bacc as bacc
import concourse.mybir as mybir
from concourse import tile
from concourse.bass_utils import run_bass_kernel_spmd
from contextlib import ExitStack

F32 = mybir.dt.float32
F32R = mybir.dt.float32r
BF16 = mybir.dt.bfloat16
I16 = mybir.dt.int16
I32 = mybir.dt.int32
F8 = mybir.dt.float8e4
AF = mybir.ActivationFunctionType
OP = mybir.AluOpType
DRM = mybir.MatmulPerfMode.DoubleRow
BF = ml_dtypes.bfloat16

N, K, NF, L = 2048, 48, 384, 3
NCORES = 8
NLOC = N // NCORES            # 256
NG = 128                      # nodes per group
TG = NG * K                   # 6144 tokens per group (k-major: t = k*128 + n)
GC = 1536                     # gather/edge chunk (tokens)
NGC = TG // GC                # 4 chunks per group
SCALE = 30.0
EPS = 1e-5
MAGIC = 0x5F3759DF

O_W0X = 0
O_W2 = 1152
O_DW0 = 2304
O_DW1 = 6912
WMC = 11520
O8_W0E = 0
O8_W1 = 2304
O8_W0N = 4608
W8C = 6912
SC_W0E = 32.0
SC_W1 = 16.0
SC_W0N = 32.0
O_B0 = 0
O_B1 = 3
O_B2 = 6
O_DB0 = 9
O_DB1 = 21

SL3 = (slice(0, 2), slice(1, 3), slice(0, 3, 2))

# single AllGather blob: per-core bytes = nf shard | wm shard | wm8 shard
NFB_B = NLOC * 384 * 2            # 196608
WMSH_B = L * 16 * WMC * 2         # 1105920
WM8SH_B = L * 16 * W8C            # 331776
SHB = NFB_B + WMSH_B + WM8SH_B    # 1634304

_NC_CACHE = {}


def _emit(act=None, layers=L):
    act = AF.Gelu if act is None else act
    nc = bacc.Bacc(num_swdge_queues=4)
    edge_p = nc.declare_dram_parameter("edge", [128, 2, NGC, 3, GC], F8,
                                       isOutput=False)
    wsh_p = nc.declare_dram_parameter("wsh", [1, SHB], mybir.dt.uint8,
                                      isOutput=False)
    gidx_p = nc.declare_dram_parameter("gidx", [128, 2, NGC, 3, 32], I16,
                                       isOutput=False)
    x0i_p = nc.declare_dram_parameter("x0i", [128, 16], I16, isOutput=False)
    wb_p = nc.declare_dram_parameter("wb", [L, 128, 24], F32, isOutput=False)
    ln_p = nc.declare_dram_parameter("lnpk", [L, 1, 1920], F32R, isOutput=False)
    b1r_p = nc.declare_dram_parameter("b1r", [L, 1, 384], F32R, isOutput=False)
    crow_p = nc.declare_dram_parameter("crow", [1, 769], F32R, isOutput=False)
    ccol_p = nc.declare_dram_parameter("ccol", [128, 8], F32R, isOutput=False)
    cstb_p = nc.declare_dram_parameter("constsb", [128, 128], BF16, isOutput=False)
    cst8_p = nc.declare_dram_parameter("consts8", [128, 2, 128], F8, isOutput=False)
    mask_p = nc.declare_dram_parameter("mask", [1, NLOC], F32, isOutput=False)
    out_p = nc.declare_dram_parameter("out_x", [128, 3, NLOC], F32, isOutput=True)

    with tile.TileContext(nc) as tc, ExitStack() as ctx:
        wpool = ctx.enter_context(tc.tile_pool(name="w", bufs=2))
        gpool = ctx.enter_context(tc.tile_pool(name="g", bufs=2))
        nfpool = ctx.enter_context(tc.tile_pool(name="nf8", bufs=1))
        epool = ctx.enter_context(tc.tile_pool(name="ep", bufs=1))
        work1 = ctx.enter_context(tc.tile_pool(name="work1", bufs=1))
        work2 = ctx.enter_context(tc.tile_pool(name="work2", bufs=2))
        xpool = ctx.enter_context(tc.tile_pool(name="xp", bufs=2))
        small = ctx.enter_context(tc.tile_pool(name="small", bufs=1))
        dram = ctx.enter_context(tc.tile_pool(name="dram", bufs=1, space="DRAM"))
        mm = ctx.enter_context(tc.tile_pool(name="mm", bufs=3, space="PSUM"))
        aggp = ctx.enter_context(tc.tile_pool(name="aggp", bufs=1, space="PSUM"))
        tpp = ctx.enter_context(tc.tile_pool(name="tpp", bufs=1, space="PSUM"))

        RG = [list(range(NCORES))]

        # --- shard staging + ONE AllGather (multiple concurrent collectives
        # complete out of order vs the shared Collectives sem -> consumers
        # could read in-flight data; a single collective has no such window)
        blob_st = dram.tile([1, SHB], mybir.dt.uint8, tag="bst")
        nc.sync.dma_start(blob_st[:], wsh_p[:])
        blob = dram.tile([NCORES, SHB], mybir.dt.uint8, tag="blob")
        nc.gpsimd.collective_compute(
            "AllGather", mybir.AluOpType.bypass, replica_groups=RG,
            ins=[blob_st[:]], outs=[blob[:]])
        flat = blob[:].rearrange("a s -> (a s)")

        # repack the nf gather table to contiguous node-major [N, 384]
        nfbfull = dram.tile([N, 384], BF16, tag="nfbf")
        for cc in range(NCORES):
            nc.sync.dma_start(
                nfbfull[cc * NLOC:(cc + 1) * NLOC, :],
                flat[cc * SHB:cc * SHB + NFB_B].bitcast(BF16)
                .rearrange("(n e) -> n e", n=NLOC))

        def wm_view(cc, l):
            o = cc * SHB + NFB_B + l * 16 * WMC * 2
            return (flat[o:o + 16 * WMC * 2].bitcast(BF16)
                    .rearrange("(q w) -> q w", q=16))

        def wm8_view(cc, l):
            o = cc * SHB + NFB_B + WMSH_B + l * 16 * W8C
            return (flat[o:o + 16 * W8C].bitcast(F8)
                    .rearrange("(q w) -> q w", q=16))

        # --- one-time loads ---
        crow = small.tile([1, 769], F32R, tag="crow")
        nc.sync.dma_start(crow[:], crow_p[:])
        ccol = small.tile([128, 8], F32R, tag="ccol")
        nc.sync.dma_start(ccol[:], ccol_p[:])
        cstb = small.tile([128, 128], BF16, tag="cstb")
        nc.sync.dma_start(cstb[:], cstb_p[:])
        cst8 = small.tile([128, 2, 128], F8, tag="cst8")
        nc.sync.dma_start(cst8[:], cst8_p[:])
        gidx = small.tile([128, 2, NGC, 3, 32], I16, tag="gidx")
        nc.sync.dma_start(gidx[:], gidx_p[:])
        x0i = small.tile([128, 16], I16, tag="x0i")
        nc.sync.dma_start(x0i[:], x0i_p[:])
        maskt = small.tile([1, NLOC], F32, tag="maskt")
        nc.sync.dma_start(maskt[:], mask_p[:])

        ones_col = ccol[:, 0:1]           # [128,1] ones (stats lhsT)
        ones_row = crow[0:1, 0:128]       # [1,128] ones
        ones512 = crow[0:1, 0:512]        # [1,512] ones
        eye_b = cstb[:]                   # [128,128] identity*SC_W0E bf16
        eye2 = cst8[:]                    # [128,2,128] identity pair fp8e4

        # x0 gathered from the repacked nf table (own 256 nodes, bf16)
        x0s = small.tile([128, 3, NLOC], BF16, tag="x0s")
        nc.gpsimd.dma_gather(
            x0s[:], nfbfull[:], x0i[:], num_idxs=NLOC, num_idxs_reg=NLOC,
            elem_size=384, transpose=True, queue_num=0)

        xg = {}      # (g) -> current residual tile [128,3,128] f32r
        xbg = {}     # bf16 copy for matmul rhs
        for g in range(2):
            xt = xpool.tile([128, 3, NG], F32R, tag=f"x{g}", name=f"x{g}")
            nc.vector.tensor_copy(xt[:].bitcast(F32),
                                  x0s[:, :, g * NG:(g + 1) * NG])
            xg[g] = xt
            xbt = xpool.tile([128, 3, NG], BF16, tag=f"xb{g}", name=f"xb{g}")
            nc.vector.tensor_copy(xbt[:], x0s[:, :, g * NG:(g + 1) * NG])
            xbg[g] = xbt

        def load_weights(l):
            wm = wpool.tile([128, WMC], BF16, tag="wm", name=f"wm{l}")
            for cc in range(NCORES):
                nc.scalar.dma_start(wm[cc * 16:(cc + 1) * 16, :],
                                    wm_view(cc, l))
            wm8 = wpool.tile([128, W8C], F8, tag="wm8", name=f"wm8{l}")
            for cc in range(NCORES):
                nc.scalar.dma_start(wm8[cc * 16:(cc + 1) * 16, :],
                                    wm8_view(cc, l))
            wb = wpool.tile([128, 24], F32, tag="wb", name=f"wb{l}")
            nc.scalar.dma_start(wb[:], wb_p[l])
            lnw = wpool.tile([1, 1920], F32R, tag="lnw", bufs=1,
                             name=f"lnw{l}")
            nc.scalar.dma_start(lnw[:], ln_p[l])
            b1r = wpool.tile([1, 384], F32R, tag="b1r", name=f"b1r{l}")
            nc.scalar.dma_start(b1r[:], b1r_p[l])
            return wm, wm8, wb, lnw, b1r

        wms = {0: load_weights(0)}

        # --- gather nf0[idx] once (24 x 512-idx), convert to e4m3 ---
        nf8 = {}
        for g in range(2):
            for c in range(NGC):
                t8 = nfpool.tile([128, 3, GC], F8, tag=f"nf8_{g}_{c}",
                                 name=f"nf8_{g}_{c}")
                for h in range(GC // 512):
                    stage = gpool.tile([128, 3, 512], BF16, tag="st",
                                       name=f"st{g}{c}{h}")
                    nc.gpsimd.dma_gather(
                        stage[:], nfbfull[:], gidx[:, g, c, h, :],
                        num_idxs=512, num_idxs_reg=512, elem_size=384,
                        transpose=True,
                        queue_num=(1 + g * NGC * 3 + c * 3 + h) % 4)
                    nc.vector.tensor_copy(
                        t8[:, :, h * 512:(h + 1) * 512], stage[:])
                nf8[(g, c)] = t8

        # --- edge features resident in SBUF, split per (g, c) ---
        et_all = epool.tile([128, 2, NGC, 3, GC], F8, tag="et")
        for g in range(2):
            for c in range(NGC):
                nc.sync.dma_start(et_all[:, g, c], edge_p[:, g, c])

        def prep_xw(l, g, wm, wb):
            """xw2 = dup2(x_g @ W0x_l + b0) bf16 [128,3,2,128]."""
            xwp = tpp.tile([128, 384], F32, tag="tp", name="xwp")
            for mt in range(3):
                for kt in range(3):
                    nc.tensor.matmul(
                        xwp[:, mt * NG:(mt + 1) * NG],
                        wm[:, O_W0X + kt * 384 + mt * 128: O_W0X + kt * 384 + (mt + 1) * 128],
                        xbg[g][:, kt, :],
                        start=(kt == 0), stop=(kt == 2))
            xw2 = work1.tile([128, 3, 2, NG], BF16, tag=f"xw2{g}",
                             name=f"xw2{g}")
            for mt in range(3):
                nc.vector.tensor_scalar(
                    xw2[:, mt, 0, :], xwp[:, mt * NG:(mt + 1) * NG],
                    wb[:, O_B0 + mt:O_B0 + mt + 1], None, op0=OP.add)
            nc.vector.tensor_copy(xw2[:, :, 1, :], xw2[:, :, 0, :])
            return xw2

        def kloop(l, g, xw2, wm8, b1r):
            """Accumulate hsum = sum_k h1 for group g. Returns PSUM tile."""
            hsum = aggp.tile([128, 384], F32, tag="agg", name=f"hs{l}{g}")
            for c in range(NGC):
                for qq in range(GC // 256):
                    off = qq * 256
                    h0g = work2.tile([128, 3, 256], F8, tag="h0g", name="h0g")
                    hp = mm.tile([128, 3, 256], F32, tag="mm", name="hp")
                    for mt in range(3):
                        for p8, sl in enumerate(SL3):
                            nc.tensor.matmul(
                                hp[:, mt, :],
                                wm8[:, O8_W0E + mt * 768 + p8 * 256:
                                    O8_W0E + mt * 768 + (p8 + 1) * 256]
                                .rearrange("p (a b) -> p a b", a=2),
                                et_all[:, g, c, sl, off:off + 256],
                                start=(p8 == 0), stop=False, perf_mode=DRM)
                        for p8, sl in enumerate(SL3):
                            nc.tensor.matmul(
                                hp[:, mt, :],
                                wm8[:, O8_W0N + mt * 768 + p8 * 256:
                                    O8_W0N + mt * 768 + (p8 + 1) * 256]
                                .rearrange("p (a b) -> p a b", a=2),
                                nf8[(g, c)][:, sl, off:off + 256],
                                start=False, stop=False, perf_mode=DRM)
                        nc.tensor.matmul(
                            hp[:, mt, :], eye_b,
                            xw2[:, mt, :, :].rearrange("p a b -> p (a b)"),
                            start=False, stop=True)
                    nc.scalar.activation(h0g[:], hp[:], act,
                                         scale=1.0 / SC_W0E)
                    h1g = work2.tile([128, 3, 256], F8, tag="h1g", name="h1g")
                    h1p = mm.tile([128, 3, 256], F32, tag="mm", name="h1p")
                    for mt in range(3):
                        nc.tensor.matmul(
                            h1p[:, mt, :],
                            b1r[0:1, mt * 128:(mt + 1) * 128],
                            ones512[0:1, 0:256], start=True, stop=False)
                        for p8, sl in enumerate(SL3):
                            nc.tensor.matmul(
                                h1p[:, mt, :],
                                wm8[:, O8_W1 + mt * 768 + p8 * 256:
                                    O8_W1 + mt * 768 + (p8 + 1) * 256]
                                .rearrange("p (a b) -> p a b", a=2),
                                h0g[:, sl, :],
                                start=False, stop=(p8 == 2), perf_mode=DRM)
                    nc.scalar.activation(h1g[:], h1p[:], act,
                                         scale=1.0 / SC_W1)
                    first = (c == 0 and qq == 0)
                    last = (c == NGC - 1 and qq == GC // 256 - 1)
                    for mt in range(3):
                        nc.tensor.matmul(
                            hsum[:, mt * NG:(mt + 1) * NG],
                            eye2,
                            h1g[:, mt, :]
                            .rearrange("p (a b) -> p a b", a=2),
                            start=(first and mt == 0),
                            stop=last,
                            perf_mode=DRM,
                            skip_group_check=True)
            return hsum

        def rsqrt_row(v):
            """[1,n] f32 SBUF -> [1,n] f32 rstd, DVE-only Newton iteration."""
            n = v.shape[-1]
            yi = small.tile([1, n], F32, tag="yi", name="yi")
            tn = small.tile([1, n], F32, tag="tn", name="tn")
            nc.vector.tensor_scalar(
                yi[:].bitcast(I32), v[:].bitcast(I32), 1, None,
                op0=OP.logical_shift_right)
            nc.vector.tensor_copy(tn[:], yi[:].bitcast(I32))
            nc.vector.tensor_scalar(tn[:], tn[:], -1.0, float(MAGIC),
                                    op0=OP.mult, op1=OP.add)
            nc.vector.tensor_copy(yi[:].bitcast(I32), tn[:])
            y = yi[:].bitcast(F32)
            for _ in range(2):
                nc.vector.tensor_mul(tn[:], y, y)
                nc.vector.tensor_mul(tn[:], tn[:], v[:])
                nc.vector.tensor_scalar(tn[:], tn[:], -0.5, 1.5,
                                        op0=OP.mult, op1=OP.add)
                nc.vector.tensor_mul(y, y, tn[:])
            return yi

        def layernorm(src, lnw, ln_i, g, masked, tag, tp_tile):
            """src: [128,3,128] F32R tile -> new [128,3,128] f32r tile."""
            maskg = maskt[0:1, g * NG:(g + 1) * NG]
            sq = work1.tile([128, 3, NG], F32R, tag="sq", name="sq")
            nc.vector.tensor_mul(sq[:], src[:].bitcast(F32), src[:].bitcast(F32))
            st = tp_tile("st")
            for kt in range(3):
                nc.tensor.matmul(st[0:1, 0:NG], ones_col, src[:, kt, :],
                                 start=(kt == 0), stop=(kt == 2))
            for kt in range(3):
                nc.tensor.matmul(st[0:1, NG:2 * NG], ones_col, sq[:, kt, :],
                                 start=(kt == 0), stop=(kt == 2))
            sm = small.tile([1, 2 * NG], F32, tag="sm", name="sm")
            nc.vector.tensor_scalar_mul(sm[:], st[0:1, 0:2 * NG], 1.0 / NF)
            var = small.tile([1, NG], F32, tag="var", name="var")
            nc.vector.tensor_mul(var[:], sm[0:1, 0:NG], sm[0:1, 0:NG])
            nc.vector.tensor_sub(var[:], sm[0:1, NG:2 * NG], var[:])
            nc.vector.tensor_scalar_add(var[:], var[:], EPS)
            rstd = rsqrt_row(var)
            rv = small.tile([1, 384], F32R, tag="rv", name="rv")
            nmr = small.tile([1, NG], F32, tag="nmr", name="nmr")
            nc.vector.tensor_scalar(nmr[:], sm[0:1, 0:NG], -1.0, None,
                                    op0=OP.mult)
            nc.vector.tensor_mul(nmr[:], nmr[:], rstd[:].bitcast(F32))
            if masked:
                nc.vector.tensor_mul(rv[0:1, 0:NG], rstd[:].bitcast(F32), maskg)
                nc.vector.tensor_mul(rv[0:1, NG:2 * NG], nmr[:], maskg)
                nc.vector.tensor_copy(rv[0:1, 2 * NG:3 * NG], maskg)
            else:
                nc.vector.tensor_copy(rv[0:1, 0:NG], rstd[:].bitcast(F32))
                nc.vector.tensor_copy(rv[0:1, NG:2 * NG], nmr[:])
                nc.vector.tensor_copy(rv[0:1, 2 * NG:3 * NG],
                                      ones_row.bitcast(F32))
            outt = xpool.tile([128, 3, NG], F32R, tag=tag, name=tag)
            stp = tp_tile("stS")
            for mt in range(3):
                woff = ln_i * 384 + mt * 128
                nc.tensor.matmul(stp[:, mt * NG:(mt + 1) * NG],
                                 lnw[0:1, woff:woff + 128],
                                 rv[0:1, 0:NG], start=True, stop=True)
            nc.vector.tensor_mul(
                outt[:], src[:].bitcast(F32),
                stp[:].rearrange("p (a b) -> p a b", a=3))
            stp2 = tp_tile("stT")
            for mt in range(3):
                woff = ln_i * 384 + mt * 128
                nc.tensor.matmul(stp2[:, mt * NG:(mt + 1) * NG],
                                 lnw[0:1, 768 + woff:768 + woff + 128],
                                 rv[0:1, 2 * NG:3 * NG], start=True, stop=False)
                nc.tensor.matmul(stp2[:, mt * NG:(mt + 1) * NG],
                                 lnw[0:1, woff:woff + 128],
                                 rv[0:1, NG:2 * NG], start=False, stop=True)
            nc.vector.tensor_add(
                outt[:], outt[:].bitcast(F32),
                stp2[:].rearrange("p (a b) -> p a b", a=3))
            return outt

        def tail(l, g, hsum, wm, wb, lnw, wm_next, wb_next, final=False):

            def tp_tile(name):
                if final:
                    t = mm.tile([128, 3, 256], F32, tag="mm", name=name)
                    return t[:].rearrange("p a b -> p (a b)")[:, 0:384]
                return tpp.tile([128, 384], F32, tag="tp", name=name)
            hsum_s = work1.tile([128, 3, NG], BF16, tag="hsum_s", name="hsum_s")
            nc.vector.tensor_copy(
                hsum_s[:], hsum[:].rearrange("p (a b) -> p a b", a=3))
            aggm = tp_tile("aggm")
            for mt in range(3):
                for kt in range(3):
                    nc.tensor.matmul(
                        aggm[:, mt * NG:(mt + 1) * NG],
                        wm[:, O_W2 + kt * 384 + mt * 128: O_W2 + kt * 384 + (mt + 1) * 128],
                        hsum_s[:, kt, :],
                        start=(kt == 0), stop=(kt == 2))
            x1p = work1.tile([128, 3, NG], F32R, tag="x1p", name="x1p")
            for mt in range(3):
                nc.vector.tensor_scalar(
                    x1p[:, mt, :], aggm[:, mt * NG:(mt + 1) * NG],
                    1.0 / SCALE, b2s[:, mt:mt + 1], op0=OP.mult, op1=OP.add)
            nc.vector.tensor_add(x1p[:], x1p[:].bitcast(F32),
                                 xg[g][:].bitcast(F32))
            x1 = layernorm(x1p, lnw, 0, g, masked=False, tag=f"x1_{g}",
                           tp_tile=tp_tile)
            x1b = work1.tile([128, 3, NG], BF16, tag=f"x1b{g}", bufs=2,
                             name="x1b")
            nc.vector.tensor_copy(x1b[:], x1[:].bitcast(F32))

            d0g = work1.tile([128, 12, NG], BF16, tag="d0g", name="d0g")
            for r in range(4):
                dp = tp_tile("dp")
                for j in range(3):
                    mt = r * 3 + j
                    reg = dp[:, j * NG:(j + 1) * NG]
                    for kt in range(3):
                        nc.tensor.matmul(
                            reg,
                            wm[:, O_DW0 + kt * 1536 + mt * 128: O_DW0 + kt * 1536 + (mt + 1) * 128],
                            x1b[:, kt, :],
                            start=(kt == 0), stop=(kt == 2))
                    nc.scalar.activation(d0g[:, mt, :], reg, act,
                                         bias=wb[:, O_DB0 + mt:O_DB0 + mt + 1])
            d1p = tp_tile("d1p")
            for mt in range(3):
                for kt in range(12):
                    nc.tensor.matmul(
                        d1p[:, mt * NG:(mt + 1) * NG],
                        wm[:, O_DW1 + kt * 384 + mt * 128: O_DW1 + kt * 384 + (mt + 1) * 128],
                        d0g[:, kt, :],
                        start=(kt == 0), stop=(kt == 11))
            x2p = work1.tile([128, 3, NG], F32R, tag="x2p", name="x2p")
            for mt in range(3):
                nc.vector.tensor_scalar(
                    x2p[:, mt, :], d1p[:, mt * NG:(mt + 1) * NG],
                    1.0, wb[:, O_DB1 + mt:O_DB1 + mt + 1],
                    op0=OP.mult, op1=OP.add)
            nc.vector.tensor_add(x2p[:], x2p[:].bitcast(F32),
                                 x1[:].bitcast(F32))
            xo = layernorm(x2p, lnw, 1, g, masked=True, tag=f"x{g}",
                           tp_tile=tp_tile)
            xg[g] = xo
            if l + 1 < layers:
                xb = xpool.tile([128, 3, NG], BF16, tag=f"xb{g}", name=f"xb{g}")
                nc.vector.tensor_copy(xb[:], xo[:].bitcast(F32))
                xbg[g] = xb
                return prep_xw(l + 1, g, wm_next, wb_next)
            nc.sync.dma_start(out_p[:, :, g * NG:(g + 1) * NG],
                              xo[:].bitcast(F32))
            return None

        # ================= pipeline =================
        wms[1] = load_weights(1)
        b2s_all = {}

        def get_b2s(l, wb):
            if l not in b2s_all:
                t = small.tile([128, 3], F32, tag=f"b2s{l % 2}", name=f"b2s{l}")
                nc.vector.tensor_scalar_mul(t[:], wb[:, O_B2:O_B2 + 3],
                                            K / SCALE)
                b2s_all[l] = t
            return b2s_all[l]

        xw2s = {}
        wm0, _, wb0, _, _ = wms[0]
        for g in range(2):
            xw2s[g] = prep_xw(0, g, wm0, wb0)

        for l in range(layers):
            wm, wm8, wb, lnw, b1r = wms[l]
            b2s = get_b2s(l, wb)
            if l + 1 < layers:
                if l + 1 not in wms:
                    wms[l + 1] = load_weights(l + 1)
                wm_next, _, wb_next, _, _ = wms[l + 1]
            else:
                wm_next = wb_next = None
            for g in range(2):
                hsum = kloop(l, g, xw2s[g], wm8, b1r)
                xw2s[g] = tail(l, g, hsum, wm, wb, lnw, wm_next, wb_next,
                               final=(l == layers - 1 and g == 1))

    nc.finalize()
    return nc


def _get_nc():
    if "nc" not in _NC_CACHE:
        _NC_CACHE["nc"] = _emit()
    return _NC_CACHE["nc"]


def _fm(w):
    """[in, out] fp32 -> [128, n_kt*out] (feature-major lhsT blob columns)."""
    i, o = w.shape
    return np.ascontiguousarray(
        w.reshape(i // 128, 128, o).transpose(1, 0, 2).reshape(128, -1))


def _wrap_idx(vals):
    """[n] int -> [128, n//16] int16 wrapped (i -> [i%16, i//16]) x8 replicas."""
    n = vals.shape[0]
    w = np.ascontiguousarray(vals.reshape(n // 16, 16).T).astype(np.int16)
    return np.tile(w, (8, 1))


def _marshal(inputs):
    nf = np.asarray(inputs["node_features"], np.float32)
    ef = np.asarray(inputs["edge_features"], np.float32)
    idx = np.asarray(inputs["neighbor_indices"])
    mask = np.asarray(inputs["mask"], np.float32)

    f8np = mybir.dt.np(mybir.dt.float8e4)
    nfb = nf.astype(BF)                                    # [N,384] full table
    wm = np.empty((L, 128, WMC), BF)
    wm8 = np.empty((L, 128, W8C), f8np)
    wb = np.empty((L, 128, 24), np.float32)
    lnpk = np.empty((L, 1, 1920), np.float32)
    b1r_m = np.empty((L, 1, 384), np.float32)
    for l in range(L):
        w0 = np.asarray(inputs["msg_w0"], np.float32)[l]
        cols = [
            _fm(w0[0:384]),
            _fm(np.asarray(inputs["msg_w2"], np.float32)[l]),
            _fm(np.asarray(inputs["dense_w0"], np.float32)[l]),
            _fm(np.asarray(inputs["dense_w1"], np.float32)[l]),
        ]
        wm[l] = np.concatenate(cols, axis=1).astype(BF)
        w0e = _fm(w0[384:768])
        w1f = _fm(np.asarray(inputs["msg_w1"], np.float32)[l])
        w0n = _fm(w0[1152:1536])
        c8 = []
        for W, sc in ((w0e, SC_W0E), (w1f, SC_W1), (w0n, SC_W0N)):
            q = (W * sc).astype(f8np)
            d = (W * sc - q.astype(np.float32)).astype(f8np)
            for mt in range(3):
                blk = lambda A, kt: A[:, kt * 384 + mt * 128:
                                      kt * 384 + (mt + 1) * 128]
                c8 += [blk(q, 0), blk(q, 1), blk(d, 1), blk(d, 2),
                       blk(d, 0), blk(q, 2)]
        wm8[l] = np.concatenate(
            [c.astype(f8np) for c in c8], axis=1)
        bcols = [
            np.asarray(inputs["msg_b0"], np.float32)[l].reshape(3, 128).T,
            np.asarray(inputs["msg_b1"], np.float32)[l].reshape(3, 128).T,
            np.asarray(inputs["msg_b2"], np.float32)[l].reshape(3, 128).T,
            np.asarray(inputs["dense_b0"], np.float32)[l].reshape(12, 128).T,
            np.asarray(inputs["dense_b1"], np.float32)[l].reshape(3, 128).T,
        ]
        wb[l] = np.concatenate(bcols, axis=1)
        lnpk[l, 0] = np.concatenate([
            np.asarray(inputs["ln1_w"], np.float32)[l],
            np.asarray(inputs["ln2_w"], np.float32)[l],
            np.asarray(inputs["ln1_b"], np.float32)[l],
            np.asarray(inputs["ln2_b"], np.float32)[l],
            np.asarray(inputs["msg_b1"], np.float32)[l] * SC_W1])
        b1r_m[l, 0] = np.asarray(inputs["msg_b1"], np.float32)[l] * SC_W1
    crow = np.ones((1, 769), np.float32)
    ccol = np.ones((128, 8), np.float32)
    constsb = (np.eye(128, dtype=np.float32) * SC_W0E).astype(BF)
    consts8 = np.broadcast_to(np.eye(128, dtype=np.float32), (2, 128, 128))
    consts8 = np.ascontiguousarray(
        consts8.transpose(1, 0, 2)).astype(f8np)

    in_maps = []
    for c in range(NCORES):
        lo = slice(c * NLOC, (c + 1) * NLOC)
        efc = ef[lo]                                       # [256,48,384]
        idc = idx[lo]                                      # [256,48]
        edge = np.empty((128, 2, NGC, 3, GC), f8np)
        gidx = np.empty((128, 2, NGC, 3, 32), np.int16)
        for g in range(2):
            gs = slice(g * NG, (g + 1) * NG)
            E = efc[gs].transpose(1, 0, 2).reshape(TG, 384)    # k-major tokens
            idx_k = np.ascontiguousarray(idc[gs].T).reshape(TG)
            for cc in range(NGC):
                Ec = E[cc * GC:(cc + 1) * GC]
                edge[:, g, cc] = (Ec.reshape(GC, 3, 128)
                                  .transpose(2, 1, 0).astype(f8np))
                for h in range(3):
                    t0 = cc * GC + h * 512
                    gidx[:, g, cc, h] = _wrap_idx(idx_k[t0:t0 + 512])
        x0i = _wrap_idx(np.arange(c * NLOC, (c + 1) * NLOC))
        wsh = np.concatenate([
            np.ascontiguousarray(nfb[lo]).view(np.uint8).reshape(-1),
            np.ascontiguousarray(
                wm[:, c * 16:(c + 1) * 16, :]).view(np.uint8).reshape(-1),
            np.ascontiguousarray(
                wm8[:, c * 16:(c + 1) * 16, :]).view(np.uint8).reshape(-1),
        ])[None, :]
        in_maps.append(dict(
            edge=edge, gidx=gidx,
            x0i=x0i, wsh=wsh,
            wb=wb, lnpk=lnpk, crow=crow, ccol=ccol,
            constsb=constsb, consts8=consts8,
            b1r=b1r_m,
            mask=np.ascontiguousarray(mask[lo])[None, :]))
    return in_maps


def _unshard(results):
    out = np.empty((N, NF), np.float32)
    for c in range(NCORES):
        xfm = results[c]["out_x"]                          # [128,3,256]
        out[c * NLOC:(c + 1) * NLOC] = xfm.transpose(2, 1, 0).reshape(NLOC, NF)
    return out


def kernel(**inputs):
    nc = _get_nc()
    in_maps = _marshal(inputs)
    res = run_bass_kernel_spmd(nc, in_maps, list(range(NCORES)), trace=False)
    return _unshard(res.results)


# revision 8
# speedup vs baseline: 1.0305x; 1.0024x over previous
"""Trainium2 Bass kernel v6: sharded weights + device AllGather.

Measured reality on this axon-tunneled setup: per-exec cost is dominated
by INPUT STAGING at ~0.85ms per MB of per-core input bytes; compute is
nearly free (L1 vs L3 ablation shows ~0.6ms/layer). AllGather of 11MB
costs ~0.9ms. So v6 ships the replicated tensors (wm bf16, wm8 dual-fp8,
nf gather table) SHARDED 1/8 per core and reassembles them on device with
three DRAM AllGathers — full numeric precision, ~11.4MB fewer input bytes
per core. x0 is cut (gathered from the assembled nf table); the big const
blob is split to two small rows.

Compute structure is v4's (gather-once nf table in e4m3, resident edges,
quarter-tile dual-fp8 message MLP, 2-group pipelined tails).
"""
import numpy as np
import ml_dtypes
import concourse.bass as bass
import concourse.bacc as bacc
import concourse.mybir as mybir
from concourse import tile
from concourse.bass_utils import run_bass_kernel_spmd
from contextlib import ExitStack

F32 = mybir.dt.float32
F32R = mybir.dt.float32r
BF16 = mybir.dt.bfloat16
I16 = mybir.dt.int16
I32 = mybir.dt.int32
F8 = mybir.dt.float8e4
AF = mybir.ActivationFunctionType
OP = mybir.AluOpType
DRM = mybir.MatmulPerfMode.DoubleRow
BF = ml_dtypes.bfloat16

N, K, NF, L = 2048, 48, 384, 3
NCORES = 8
NLOC = N // NCORES            # 256
NG = 128                      # nodes per group
TG = NG * K                   # 6144 tokens per group (k-major: t = k*128 + n)
GC = 1536                     # gather/edge chunk (tokens)
NGC = TG // GC                # 4 chunks per group
SCALE = 30.0
EPS = 1e-5
MAGIC = 0x5F3759DF

O_W0X = 0
O_W2 = 1152
O_DW0 = 2304
O_DW1 = 6912
WMC = 11520
O8_W0E = 0
O8_W1 = 2304
O8_W0N = 4608
W8C = 6912
SC_W0E = 32.0
SC_W1 = 16.0
SC_W0N = 32.0
O_B0 = 0
O_B1 = 3
O_B2 = 6
O_DB0 = 9
O_DB1 = 21

SL3 = (slice(0, 2), slice(1, 3), slice(0, 3, 2))

# single AllGather blob: per-core bytes = nf shard | wm shard | wm8 shard
NFB_B = NLOC * 384 * 2            # 196608
WMSH_B = L * 16 * WMC * 2         # 1105920
WM8SH_B = L * 16 * W8C            # 331776
SHB = NFB_B + WMSH_B + WM8SH_B    # 1634304

_NC_CACHE = {}


def _emit(act=None, layers=L):
    act = AF.Gelu if act is None else act
    nc = bacc.Bacc(num_swdge_queues=4)
    edge_p = nc.declare_dram_parameter("edge", [128, 2, NGC, 3, GC], F8,
                                       isOutput=False)
    wsh_p = nc.declare_dram_parameter("wsh", [1, SHB], mybir.dt.uint8,
                                      isOutput=False)
    gidx_p = nc.declare_dram_parameter("gidx", [128, 2, NGC, 3, 32], I16,
                                       isOutput=False)
    x0i_p = nc.declare_dram_parameter("x0i", [128, 16], I16, isOutput=False)
    wb_p = nc.declare_dram_parameter("wb", [L, 128, 24], F32, isOutput=False)
    ln_p = nc.declare_dram_parameter("lnpk", [L, 1, 1920], F32R, isOutput=False)
    b1r_p = nc.declare_dram_parameter("b1r", [L, 1, 384], F32R, isOutput=False)
    crow_p = nc.declare_dram_parameter("crow", [1, 769], F32R, isOutput=False)
    ccol_p = nc.declare_dram_parameter("ccol", [128, 8], F32R, isOutput=False)
    cstb_p = nc.declare_dram_parameter("constsb", [128, 128], BF16, isOutput=False)
    cst8_p = nc.declare_dram_parameter("consts8", [128, 2, 128], F8, isOutput=False)
    mask_p = nc.declare_dram_parameter("mask", [1, NLOC], F32, isOutput=False)
    out_p = nc.declare_dram_parameter("out_x", [128, 3, NLOC], BF16, isOutput=True)

    with tile.TileContext(nc) as tc, ExitStack() as ctx:
        wpool = ctx.enter_context(tc.tile_pool(name="w", bufs=2))
        gpool = ctx.enter_context(tc.tile_pool(name="g", bufs=2))
        nfpool = ctx.enter_context(tc.tile_pool(name="nf8", bufs=1))
        epool = ctx.enter_context(tc.tile_pool(name="ep", bufs=1))
        work1 = ctx.enter_context(tc.tile_pool(name="work1", bufs=1))
        work2 = ctx.enter_context(tc.tile_pool(name="work2", bufs=2))
        xpool = ctx.enter_context(tc.tile_pool(name="xp", bufs=2))
        small = ctx.enter_context(tc.tile_pool(name="small", bufs=1))
        dram = ctx.enter_context(tc.tile_pool(name="dram", bufs=1, space="DRAM"))
        mm = ctx.enter_context(tc.tile_pool(name="mm", bufs=3, space="PSUM"))
        aggp = ctx.enter_context(tc.tile_pool(name="aggp", bufs=1, space="PSUM"))
        tpp = ctx.enter_context(tc.tile_pool(name="tpp", bufs=1, space="PSUM"))

        RG = [list(range(NCORES))]

        # --- shard staging + ONE AllGather (multiple concurrent collectives
        # complete out of order vs the shared Collectives sem -> consumers
        # could read in-flight data; a single collective has no such window)
        blob_st = dram.tile([1, SHB], mybir.dt.uint8, tag="bst")
        nc.sync.dma_start(blob_st[:], wsh_p[:])
        blob = dram.tile([NCORES, SHB], mybir.dt.uint8, tag="blob",
                         addr_space="Shared")
        nc.gpsimd.collective_compute(
            "AllGather", mybir.AluOpType.bypass, replica_groups=RG,
            ins=[blob_st[:]], outs=[blob[:]])
        flat = blob[:].rearrange("a s -> (a s)")

        # repack the nf gather table to contiguous node-major [N, 384]
        nfbfull = dram.tile([N, 384], BF16, tag="nfbf")
        for cc in range(NCORES):
            nc.sync.dma_start(
                nfbfull[cc * NLOC:(cc + 1) * NLOC, :],
                flat[cc * SHB:cc * SHB + NFB_B].bitcast(BF16)
                .rearrange("(n e) -> n e", n=NLOC))

        def wm_view(cc, l):
            o = cc * SHB + NFB_B + l * 16 * WMC * 2
            return (flat[o:o + 16 * WMC * 2].bitcast(BF16)
                    .rearrange("(q w) -> q w", q=16))

        def wm8_view(cc, l):
            o = cc * SHB + NFB_B + WMSH_B + l * 16 * W8C
            return (flat[o:o + 16 * W8C].bitcast(F8)
                    .rearrange("(q w) -> q w", q=16))

        # --- one-time loads ---
        crow = small.tile([1, 769], F32R, tag="crow")
        nc.sync.dma_start(crow[:], crow_p[:])
        ccol = small.tile([128, 8], F32R, tag="ccol")
        nc.sync.dma_start(ccol[:], ccol_p[:])
        cstb = small.tile([128, 128], BF16, tag="cstb")
        nc.sync.dma_start(cstb[:], cstb_p[:])
        cst8 = small.tile([128, 2, 128], F8, tag="cst8")
        nc.sync.dma_start(cst8[:], cst8_p[:])
        gidx = small.tile([128, 2, NGC, 3, 32], I16, tag="gidx")
        nc.sync.dma_start(gidx[:], gidx_p[:])
        x0i = small.tile([128, 16], I16, tag="x0i")
        nc.sync.dma_start(x0i[:], x0i_p[:])
        maskt = small.tile([1, NLOC], F32, tag="maskt")
        nc.sync.dma_start(maskt[:], mask_p[:])

        ones_col = ccol[:, 0:1]           # [128,1] ones (stats lhsT)
        ones_row = crow[0:1, 0:128]       # [1,128] ones
        ones512 = crow[0:1, 0:512]        # [1,512] ones
        eye_b = cstb[:]                   # [128,128] identity*SC_W0E bf16
        eye2 = cst8[:]                    # [128,2,128] identity pair fp8e4

        # x0 gathered from the repacked nf table (own 256 nodes, bf16)
        x0s = small.tile([128, 3, NLOC], BF16, tag="x0s")
        nc.gpsimd.dma_gather(
            x0s[:], nfbfull[:], x0i[:], num_idxs=NLOC, num_idxs_reg=NLOC,
            elem_size=384, transpose=True, queue_num=0)

        xg = {}      # (g) -> current residual tile [128,3,128] f32r
        xbg = {}     # bf16 copy for matmul rhs
        for g in range(2):
            xt = xpool.tile([128, 3, NG], F32R, tag=f"x{g}", name=f"x{g}")
            nc.vector.tensor_copy(xt[:].bitcast(F32),
                                  x0s[:, :, g * NG:(g + 1) * NG])
            xg[g] = xt
            xbt = xpool.tile([128, 3, NG], BF16, tag=f"xb{g}", name=f"xb{g}")
            nc.vector.tensor_copy(xbt[:], x0s[:, :, g * NG:(g + 1) * NG])
            xbg[g] = xbt

        def load_weights(l):
            wm = wpool.tile([128, WMC], BF16, tag="wm", name=f"wm{l}")
            for cc in range(NCORES):
                nc.scalar.dma_start(wm[cc * 16:(cc + 1) * 16, :],
                                    wm_view(cc, l))
            wm8 = wpool.tile([128, W8C], F8, tag="wm8", name=f"wm8{l}")
            for cc in range(NCORES):
                nc.scalar.dma_start(wm8[cc * 16:(cc + 1) * 16, :],
                                    wm8_view(cc, l))
            wb = wpool.tile([128, 24], F32, tag="wb", name=f"wb{l}")
            nc.scalar.dma_start(wb[:], wb_p[l])
            lnw = wpool.tile([1, 1920], F32R, tag="lnw", bufs=1,
                             name=f"lnw{l}")
            nc.scalar.dma_start(lnw[:], ln_p[l])
            b1r = wpool.tile([1, 384], F32R, tag="b1r", name=f"b1r{l}")
            nc.scalar.dma_start(b1r[:], b1r_p[l])
            return wm, wm8, wb, lnw, b1r

        wms = {0: load_weights(0)}

        # --- gather nf0[idx] once (24 x 512-idx), convert to e4m3 ---
        nf8 = {}
        for g in range(2):
            for c in range(NGC):
                t8 = nfpool.tile([128, 3, GC], F8, tag=f"nf8_{g}_{c}",
                                 name=f"nf8_{g}_{c}")
                for h in range(GC // 512):
                    stage = gpool.tile([128, 3, 512], BF16, tag="st",
                                       name=f"st{g}{c}{h}")
                    nc.gpsimd.dma_gather(
                        stage[:], nfbfull[:], gidx[:, g, c, h, :],
                        num_idxs=512, num_idxs_reg=512, elem_size=384,
                        transpose=True,
                        queue_num=(1 + g * NGC * 3 + c * 3 + h) % 4)
                    nc.vector.tensor_copy(
                        t8[:, :, h * 512:(h + 1) * 512], stage[:])
                nf8[(g, c)] = t8

        # --- edge features resident in SBUF, split per (g, c) ---
        et_all = epool.tile([128, 2, NGC, 3, GC], F8, tag="et")
        for g in range(2):
            for c in range(NGC):
                nc.sync.dma_start(et_all[:, g, c], edge_p[:, g, c])

        def prep_xw(l, g, wm, wb):
            """xw2 = dup2(x_g @ W0x_l + b0) bf16 [128,3,2,128]."""
            xwp = tpp.tile([128, 384], F32, tag="tp", name="xwp")
            for mt in range(3):
                for kt in range(3):
                    nc.tensor.matmul(
                        xwp[:, mt * NG:(mt + 1) * NG],
                        wm[:, O_W0X + kt * 384 + mt * 128: O_W0X + kt * 384 + (mt + 1) * 128],
                        xbg[g][:, kt, :],
                        start=(kt == 0), stop=(kt == 2))
            xw2 = work1.tile([128, 3, 2, NG], BF16, tag=f"xw2{g}",
                             name=f"xw2{g}")
            for mt in range(3):
                nc.vector.tensor_scalar(
                    xw2[:, mt, 0, :], xwp[:, mt * NG:(mt + 1) * NG],
                    wb[:, O_B0 + mt:O_B0 + mt + 1], None, op0=OP.add)
            nc.vector.tensor_copy(xw2[:, :, 1, :], xw2[:, :, 0, :])
            return xw2

        def kloop(l, g, xw2, wm8, b1r):
            """Accumulate hsum = sum_k h1 for group g. Returns PSUM tile."""
            hsum = aggp.tile([128, 384], F32, tag="agg", name=f"hs{l}{g}")
            for c in range(NGC):
                for qq in range(GC // 256):
                    off = qq * 256
                    h0g = work2.tile([128, 3, 256], F8, tag="h0g", name="h0g")
                    hp = mm.tile([128, 3, 256], F32, tag="mm", name="hp")
                    for mt in range(3):
                        for p8, sl in enumerate(SL3):
                            nc.tensor.matmul(
                                hp[:, mt, :],
                                wm8[:, O8_W0E + mt * 768 + p8 * 256:
                                    O8_W0E + mt * 768 + (p8 + 1) * 256]
                                .rearrange("p (a b) -> p a b", a=2),
                                et_all[:, g, c, sl, off:off + 256],
                                start=(p8 == 0), stop=False, perf_mode=DRM)
                        for p8, sl in enumerate(SL3):
                            nc.tensor.matmul(
                                hp[:, mt, :],
                                wm8[:, O8_W0N + mt * 768 + p8 * 256:
                                    O8_W0N + mt * 768 + (p8 + 1) * 256]
                                .rearrange("p (a b) -> p a b", a=2),
                                nf8[(g, c)][:, sl, off:off + 256],
                                start=False, stop=False, perf_mode=DRM)
                        nc.tensor.matmul(
                            hp[:, mt, :], eye_b,
                            xw2[:, mt, :, :].rearrange("p a b -> p (a b)"),
                            start=False, stop=True)
                    nc.scalar.activation(h0g[:], hp[:], act,
                                         scale=1.0 / SC_W0E)
                    h1g = work2.tile([128, 3, 256], F8, tag="h1g", name="h1g")
                    h1p = mm.tile([128, 3, 256], F32, tag="mm", name="h1p")
                    for mt in range(3):
                        nc.tensor.matmul(
                            h1p[:, mt, :],
                            b1r[0:1, mt * 128:(mt + 1) * 128],
                            ones512[0:1, 0:256], start=True, stop=False)
                        for p8, sl in enumerate(SL3):
                            nc.tensor.matmul(
                                h1p[:, mt, :],
                                wm8[:, O8_W1 + mt * 768 + p8 * 256:
                                    O8_W1 + mt * 768 + (p8 + 1) * 256]
                                .rearrange("p (a b) -> p a b", a=2),
                                h0g[:, sl, :],
                                start=False, stop=(p8 == 2), perf_mode=DRM)
                    nc.scalar.activation(h1g[:], h1p[:], act,
                                         scale=1.0 / SC_W1)
                    first = (c == 0 and qq == 0)
                    last = (c == NGC - 1 and qq == GC // 256 - 1)
                    for mt in range(3):
                        nc.tensor.matmul(
                            hsum[:, mt * NG:(mt + 1) * NG],
                            eye2,
                            h1g[:, mt, :]
                            .rearrange("p (a b) -> p a b", a=2),
                            start=(first and mt == 0),
                            stop=last,
                            perf_mode=DRM,
                            skip_group_check=True)
            return hsum

        def rsqrt_row(v):
            """[1,n] f32 SBUF -> [1,n] f32 rstd, DVE-only Newton iteration."""
            n = v.shape[-1]
            yi = small.tile([1, n], F32, tag="yi", name="yi")
            tn = small.tile([1, n], F32, tag="tn", name="tn")
            nc.vector.tensor_scalar(
                yi[:].bitcast(I32), v[:].bitcast(I32), 1, None,
                op0=OP.logical_shift_right)
            nc.vector.tensor_copy(tn[:], yi[:].bitcast(I32))
            nc.vector.tensor_scalar(tn[:], tn[:], -1.0, float(MAGIC),
                                    op0=OP.mult, op1=OP.add)
            nc.vector.tensor_copy(yi[:].bitcast(I32), tn[:])
            y = yi[:].bitcast(F32)
            for _ in range(2):
                nc.vector.tensor_mul(tn[:], y, y)
                nc.vector.tensor_mul(tn[:], tn[:], v[:])
                nc.vector.tensor_scalar(tn[:], tn[:], -0.5, 1.5,
                                        op0=OP.mult, op1=OP.add)
                nc.vector.tensor_mul(y, y, tn[:])
            return yi

        def layernorm(src, lnw, ln_i, g, masked, tag, tp_tile):
            """src: [128,3,128] F32R tile -> new [128,3,128] f32r tile."""
            maskg = maskt[0:1, g * NG:(g + 1) * NG]
            sq = work1.tile([128, 3, NG], F32R, tag="sq", name="sq")
            nc.vector.tensor_mul(sq[:], src[:].bitcast(F32), src[:].bitcast(F32))
            st = tp_tile("st")
            for kt in range(3):
                nc.tensor.matmul(st[0:1, 0:NG], ones_col, src[:, kt, :],
                                 start=(kt == 0), stop=(kt == 2))
            for kt in range(3):
                nc.tensor.matmul(st[0:1, NG:2 * NG], ones_col, sq[:, kt, :],
                                 start=(kt == 0), stop=(kt == 2))
            sm = small.tile([1, 2 * NG], F32, tag="sm", name="sm")
            nc.vector.tensor_scalar_mul(sm[:], st[0:1, 0:2 * NG], 1.0 / NF)
            var = small.tile([1, NG], F32, tag="var", name="var")
            nc.vector.tensor_mul(var[:], sm[0:1, 0:NG], sm[0:1, 0:NG])
            nc.vector.tensor_sub(var[:], sm[0:1, NG:2 * NG], var[:])
            nc.vector.tensor_scalar_add(var[:], var[:], EPS)
            rstd = rsqrt_row(var)
            rv = small.tile([1, 384], F32R, tag="rv", name="rv")
            nmr = small.tile([1, NG], F32, tag="nmr", name="nmr")
            nc.vector.tensor_scalar(nmr[:], sm[0:1, 0:NG], -1.0, None,
                                    op0=OP.mult)
            nc.vector.tensor_mul(nmr[:], nmr[:], rstd[:].bitcast(F32))
            if masked:
                nc.vector.tensor_mul(rv[0:1, 0:NG], rstd[:].bitcast(F32), maskg)
                nc.vector.tensor_mul(rv[0:1, NG:2 * NG], nmr[:], maskg)
                nc.vector.tensor_copy(rv[0:1, 2 * NG:3 * NG], maskg)
            else:
                nc.vector.tensor_copy(rv[0:1, 0:NG], rstd[:].bitcast(F32))
                nc.vector.tensor_copy(rv[0:1, NG:2 * NG], nmr[:])
                nc.vector.tensor_copy(rv[0:1, 2 * NG:3 * NG],
                                      ones_row.bitcast(F32))
            outt = xpool.tile([128, 3, NG], F32R, tag=tag, name=tag)
            stp = tp_tile("stS")
            for mt in range(3):
                woff = ln_i * 384 + mt * 128
                nc.tensor.matmul(stp[:, mt * NG:(mt + 1) * NG],
                                 lnw[0:1, woff:woff + 128],
                                 rv[0:1, 0:NG], start=True, stop=True)
            nc.vector.tensor_mul(
                outt[:], src[:].bitcast(F32),
                stp[:].rearrange("p (a b) -> p a b", a=3))
            stp2 = tp_tile("stT")
            for mt in range(3):
                woff = ln_i * 384 + mt * 128
                nc.tensor.matmul(stp2[:, mt * NG:(mt + 1) * NG],
                                 lnw[0:1, 768 + woff:768 + woff + 128],
                                 rv[0:1, 2 * NG:3 * NG], start=True, stop=False)
                nc.tensor.matmul(stp2[:, mt * NG:(mt + 1) * NG],
                                 lnw[0:1, woff:woff + 128],
                                 rv[0:1, NG:2 * NG], start=False, stop=True)
            nc.vector.tensor_add(
                outt[:], outt[:].bitcast(F32),
                stp2[:].rearrange("p (a b) -> p a b", a=3))
            return outt

        def tail(l, g, hsum, wm, wb, lnw, wm_next, wb_next, final=False):

            def tp_tile(name):
                if final:
                    t = mm.tile([128, 3, 256], F32, tag="mm", name=name)
                    return t[:].rearrange("p a b -> p (a b)")[:, 0:384]
                return tpp.tile([128, 384], F32, tag="tp", name=name)
            hsum_s = work1.tile([128, 3, NG], BF16, tag="hsum_s", name="hsum_s")
            nc.vector.tensor_copy(
                hsum_s[:], hsum[:].rearrange("p (a b) -> p a b", a=3))
            aggm = tp_tile("aggm")
            for mt in range(3):
                for kt in range(3):
                    nc.tensor.matmul(
                        aggm[:, mt * NG:(mt + 1) * NG],
                        wm[:, O_W2 + kt * 384 + mt * 128: O_W2 + kt * 384 + (mt + 1) * 128],
                        hsum_s[:, kt, :],
                        start=(kt == 0), stop=(kt == 2))
            x1p = work1.tile([128, 3, NG], F32R, tag="x1p", name="x1p")
            for mt in range(3):
                nc.vector.tensor_scalar(
                    x1p[:, mt, :], aggm[:, mt * NG:(mt + 1) * NG],
                    1.0 / SCALE, b2s[:, mt:mt + 1], op0=OP.mult, op1=OP.add)
            nc.vector.tensor_add(x1p[:], x1p[:].bitcast(F32),
                                 xg[g][:].bitcast(F32))
            x1 = layernorm(x1p, lnw, 0, g, masked=False, tag=f"x1_{g}",
                           tp_tile=tp_tile)
            x1b = work1.tile([128, 3, NG], BF16, tag=f"x1b{g}", bufs=2,
                             name="x1b")
            nc.vector.tensor_copy(x1b[:], x1[:].bitcast(F32))

            d0g = work1.tile([128, 12, NG], BF16, tag="d0g", name="d0g")
            for r in range(4):
                dp = tp_tile("dp")
                for j in range(3):
                    mt = r * 3 + j
                    reg = dp[:, j * NG:(j + 1) * NG]
                    for kt in range(3):
                        nc.tensor.matmul(
                            reg,
                            wm[:, O_DW0 + kt * 1536 + mt * 128: O_DW0 + kt * 1536 + (mt + 1) * 128],
                            x1b[:, kt, :],
                            start=(kt == 0), stop=(kt == 2))
                    nc.scalar.activation(d0g[:, mt, :], reg, act,
                                         bias=wb[:, O_DB0 + mt:O_DB0 + mt + 1])
            d1p = tp_tile("d1p")
            for mt in range(3):
                for kt in range(12):
                    nc.tensor.matmul(
                        d1p[:, mt * NG:(mt + 1) * NG],
                        wm[:, O_DW1 + kt * 384 + mt * 128: O_DW1 + kt * 384 + (mt + 1) * 128],
                        d0g[:, kt, :],
                        start=(kt == 0), stop=(kt == 11))
            x2p = work1.tile([128, 3, NG], F32R, tag="x2p", name="x2p")
            for mt in range(3):
                nc.vector.tensor_scalar(
                    x2p[:, mt, :], d1p[:, mt * NG:(mt + 1) * NG],
                    1.0, wb[:, O_DB1 + mt:O_DB1 + mt + 1],
                    op0=OP.mult, op1=OP.add)
            nc.vector.tensor_add(x2p[:], x2p[:].bitcast(F32),
                                 x1[:].bitcast(F32))
            xo = layernorm(x2p, lnw, 1, g, masked=True, tag=f"x{g}",
                           tp_tile=tp_tile)
            xg[g] = xo
            if l + 1 < layers:
                xb = xpool.tile([128, 3, NG], BF16, tag=f"xb{g}", name=f"xb{g}")
                nc.vector.tensor_copy(xb[:], xo[:].bitcast(F32))
                xbg[g] = xb
                return prep_xw(l + 1, g, wm_next, wb_next)
            xob = xpool.tile([128, 3, NG], BF16, tag=f"xb{g}",
                             name=f"xob{g}")
            nc.vector.tensor_copy(xob[:], xo[:].bitcast(F32))
            nc.sync.dma_start(out_p[:, :, g * NG:(g + 1) * NG], xob[:])
            return None

        # ================= pipeline =================
        wms[1] = load_weights(1)
        b2s_all = {}

        def get_b2s(l, wb):
            if l not in b2s_all:
                t = small.tile([128, 3], F32, tag=f"b2s{l % 2}", name=f"b2s{l}")
                nc.vector.tensor_scalar_mul(t[:], wb[:, O_B2:O_B2 + 3],
                                            K / SCALE)
                b2s_all[l] = t
            return b2s_all[l]

        xw2s = {}
        wm0, _, wb0, _, _ = wms[0]
        for g in range(2):
            xw2s[g] = prep_xw(0, g, wm0, wb0)

        for l in range(layers):
            wm, wm8, wb, lnw, b1r = wms[l]
            b2s = get_b2s(l, wb)
            if l + 1 < layers:
                if l + 1 not in wms:
                    wms[l + 1] = load_weights(l + 1)
                wm_next, _, wb_next, _, _ = wms[l + 1]
            else:
                wm_next = wb_next = None
            for g in range(2):
                hsum = kloop(l, g, xw2s[g], wm8, b1r)
                xw2s[g] = tail(l, g, hsum, wm, wb, lnw, wm_next, wb_next,
                               final=(l == layers - 1 and g == 1))

    nc.finalize()
    return nc


def _get_nc():
    if "nc" not in _NC_CACHE:
        _NC_CACHE["nc"] = _emit()
    return _NC_CACHE["nc"]


def _fm(w):
    """[in, out] fp32 -> [128, n_kt*out] (feature-major lhsT blob columns)."""
    i, o = w.shape
    return np.ascontiguousarray(
        w.reshape(i // 128, 128, o).transpose(1, 0, 2).reshape(128, -1))


def _wrap_idx(vals):
    """[n] int -> [128, n//16] int16 wrapped (i -> [i%16, i//16]) x8 replicas."""
    n = vals.shape[0]
    w = np.ascontiguousarray(vals.reshape(n // 16, 16).T).astype(np.int16)
    return np.tile(w, (8, 1))


def _marshal(inputs):
    nf = np.asarray(inputs["node_features"], np.float32)
    ef = np.asarray(inputs["edge_features"], np.float32)
    idx = np.asarray(inputs["neighbor_indices"])
    mask = np.asarray(inputs["mask"], np.float32)

    f8np = mybir.dt.np(mybir.dt.float8e4)
    nfb = nf.astype(BF)                                    # [N,384] full table
    wm = np.empty((L, 128, WMC), BF)
    wm8 = np.empty((L, 128, W8C), f8np)
    wb = np.empty((L, 128, 24), np.float32)
    lnpk = np.empty((L, 1, 1920), np.float32)
    b1r_m = np.empty((L, 1, 384), np.float32)
    for l in range(L):
        w0 = np.asarray(inputs["msg_w0"], np.float32)[l]
        cols = [
            _fm(w0[0:384]),
            _fm(np.asarray(inputs["msg_w2"], np.float32)[l]),
            _fm(np.asarray(inputs["dense_w0"], np.float32)[l]),
            _fm(np.asarray(inputs["dense_w1"], np.float32)[l]),
        ]
        wm[l] = np.concatenate(cols, axis=1).astype(BF)
        w0e = _fm(w0[384:768])
        w1f = _fm(np.asarray(inputs["msg_w1"], np.float32)[l])
        w0n = _fm(w0[1152:1536])
        c8 = []
        for W, sc in ((w0e, SC_W0E), (w1f, SC_W1), (w0n, SC_W0N)):
            q = (W * sc).astype(f8np)
            d = (W * sc - q.astype(np.float32)).astype(f8np)
            for mt in range(3):
                blk = lambda A, kt: A[:, kt * 384 + mt * 128:
                                      kt * 384 + (mt + 1) * 128]
                c8 += [blk(q, 0), blk(q, 1), blk(d, 1), blk(d, 2),
                       blk(d, 0), blk(q, 2)]
        wm8[l] = np.concatenate(
            [c.astype(f8np) for c in c8], axis=1)
        bcols = [
            np.asarray(inputs["msg_b0"], np.float32)[l].reshape(3, 128).T,
            np.asarray(inputs["msg_b1"], np.float32)[l].reshape(3, 128).T,
            np.asarray(inputs["msg_b2"], np.float32)[l].reshape(3, 128).T,
            np.asarray(inputs["dense_b0"], np.float32)[l].reshape(12, 128).T,
            np.asarray(inputs["dense_b1"], np.float32)[l].reshape(3, 128).T,
        ]
        wb[l] = np.concatenate(bcols, axis=1)
        lnpk[l, 0] = np.concatenate([
            np.asarray(inputs["ln1_w"], np.float32)[l],
            np.asarray(inputs["ln2_w"], np.float32)[l],
            np.asarray(inputs["ln1_b"], np.float32)[l],
            np.asarray(inputs["ln2_b"], np.float32)[l],
            np.asarray(inputs["msg_b1"], np.float32)[l] * SC_W1])
        b1r_m[l, 0] = np.asarray(inputs["msg_b1"], np.float32)[l] * SC_W1
    crow = np.ones((1, 769), np.float32)
    ccol = np.ones((128, 8), np.float32)
    constsb = (np.eye(128, dtype=np.float32) * SC_W0E).astype(BF)
    consts8 = np.broadcast_to(np.eye(128, dtype=np.float32), (2, 128, 128))
    consts8 = np.ascontiguousarray(
        consts8.transpose(1, 0, 2)).astype(f8np)

    in_maps = []
    for c in range(NCORES):
        lo = slice(c * NLOC, (c + 1) * NLOC)
        efc = ef[lo]                                       # [256,48,384]
        idc = idx[lo]                                      # [256,48]
        edge = np.empty((128, 2, NGC, 3, GC), f8np)
        gidx = np.empty((128, 2, NGC, 3, 32), np.int16)
        for g in range(2):
            gs = slice(g * NG, (g + 1) * NG)
            E = efc[gs].transpose(1, 0, 2).reshape(TG, 384)    # k-major tokens
            idx_k = np.ascontiguousarray(idc[gs].T).reshape(TG)
            for cc in range(NGC):
                Ec = E[cc * GC:(cc + 1) * GC]
                edge[:, g, cc] = (Ec.reshape(GC, 3, 128)
                                  .transpose(2, 1, 0).astype(f8np))
                for h in range(3):
                    t0 = cc * GC + h * 512
                    gidx[:, g, cc, h] = _wrap_idx(idx_k[t0:t0 + 512])
        x0i = _wrap_idx(np.arange(c * NLOC, (c + 1) * NLOC))
        wsh = np.concatenate([
            np.ascontiguousarray(nfb[lo]).view(np.uint8).reshape(-1),
            np.ascontiguousarray(
                wm[:, c * 16:(c + 1) * 16, :]).view(np.uint8).reshape(-1),
            np.ascontiguousarray(
                wm8[:, c * 16:(c + 1) * 16, :]).view(np.uint8).reshape(-1),
        ])[None, :]
        in_maps.append(dict(
            edge=edge, gidx=gidx,
            x0i=x0i, wsh=wsh,
            wb=wb, lnpk=lnpk, crow=crow, ccol=ccol,
            constsb=constsb, consts8=consts8,
            b1r=b1r_m,
            mask=np.ascontiguousarray(mask[lo])[None, :]))
    return in_maps


def _unshard(results):
    out = np.empty((N, NF), np.float32)
    for c in range(NCORES):
        xfm = results[c]["out_x"].astype(np.float32)       # [128,3,256]
        out[c * NLOC:(c + 1) * NLOC] = xfm.transpose(2, 1, 0).reshape(NLOC, NF)
    return out


def kernel(**inputs):
    nc = _get_nc()
    in_maps = _marshal(inputs)
    res = run_bass_kernel_spmd(nc, in_maps, list(range(NCORES)), trace=False)
    return _unshard(res.results)
